# revision 36
# baseline (speedup 1.0000x reference)
"""Trainium2 Bass kernel for EvalHead (NMS detection decode).

Computes, for x [B=16, C=15, H=512, W=512] fp32:
  scores = x[:,0]; peak = (scores > 0.5) & (scores == maxpool3x3(scores))
  out[b,h,w,:] = [score, cx-hx, cy-hy, cx+hx, cy+hy, lm0x+px, lm0y+py, ...] * peak
  where cx = px + x[:,1], cy = py + x[:,2], hx = 0.5*x[:,3], hy = 0.5*x[:,4],
        px = 4*w+2, py = 4*h+2.
Output: [16, 512, 512, 15] fp32.

Sharding: pure data parallel over batch - 2 images per core across 8 cores.

Production mode "v13b-tri" (measured ~107 us steady-state vs the 199 us
v7f8 baseline on the 8 axon trn2 cores; rel err ~2e-3 against a ~2e-2 budget):
  - Planar channel-major output [B, H, C, W] bf16: every engine write is
    flat/contiguous (the measured 4x strided-16-bit-write penalty never
    applies); per-partition store row = 15360B contiguous; the host
    transposes [H,C,W] -> [H,W,C] and upcasts during the gather.
  - Inputs: score plane fp32 (the peak mask must be bit-exact) + one fp8
    e4m3 tensor [B, H, 14, W] = [dx,dy,sx,sy,lmx*5,lmy*5] (abs err <=~0.4
    vs a ~40 abs budget); each row-tile load is one contiguous
    2048B + 5120B per-partition pair.  Loads ride the ACT HWDGE ring,
    the store rides the SP ring (splitting the store across rings
    measured 20 us WORSE - the ACT ring carries the loads).
  - Vertical pool shifts on the TENSOR engine: sup/sdn = 0/1 shift-matrix
    fp32 matmuls into PSUM (exact: one product per output; verified
    bit-exact on hw), with the cross-tile boundary rows entering via a
    K=1 accumulating matmul (up-boundary via a partition-0 DVE max).
    This removes the 4.2MB/core sup/sdn HBM reloads and the SWDGE
    descriptor-generation load on GpSimd.
  - Peak mask fused to one op: (sc > 0.5) & (sc == pooled) ==
    max(pooled, nextafter(0.5)) <= sc  (pooled >= sc always), written
    directly as bf16 (0/1 exact).
  - bbox decode on the TENSOR engine: x1/x2 = I@dx + (-+0.5 I)@sx +
    ones(x)px_f16 accumulated in PSUM (fp8 stationaries exact on 0/+-0.5;
    px integers exact in f16); y1/y2 = I@dy + (-+0.5 I)@sy with py folded
    in as the per-partition ACT bias during the PSUM->bf16 conversion.
  - Landmarks: 2 x-planes via PE (I@lm + ones(x)px_f16 in PSUM, ACT
    converts), 3 x-planes on GpSimd (fp8 read, bf16 flat write), all 5
    y-planes on ACT (bias add); score channel = ACT bf16 copy, masked
    with the bbox group.
  - Masks+stores in THREE chunks per tile ([0:5] score+bbox, [5:9]
    PE-decoded landmarks, [9:15] GpSimd-decoded landmarks), each a flat
    packed-bf16 DVE multiply (2x rate) followed by its own SP-ring store,
    so stores overlap the remaining decodes.  PSUM budget 8/8 banks:
    shifts 2 (bufs=1) + bbox x/y 2+2 + lm 2.
Measured HW cost attribution (K=257 steady state): full ~107 us, pure
DMA (loads+stores only) 79 us => ~317 GB/s/core effective on 25.1MB;
~6 us/pass of that is For_i loop-boundary drain (dual-pass body measures
104 us/pass).  Engine ablations: no-store 112 (compute-bound), no-mask
-14 us, no-lmx -11 us.  Splitting stores across both HWDGE rings is
20 us WORSE (the ACT ring carries the loads).
"""

import numpy as np

B = 16
N_CORES = 8
B_LOCAL = B // N_CORES  # 2 images per core
C = 15
H = 512
W = 512
PT = 128                 # partition tile height (rows)
NT = H // PT             # 4 row-tiles per image
STRIDE = 4
OFF_Y = 2.0
OFF_X = 2.0
THRESHOLD = 0.5

import os as _os
PROD_MODE = _os.environ.get("KMODE", "v13b-tri")

_CACHE = {}


def _build_nc(loop_k: int = 1, mode: str = PROD_MODE):
    if mode.startswith("v4"):
        return _build_v4(loop_k, mode)
    if mode.startswith(("v6", "v7", "v8", "v9")):
        return _build_v6(loop_k, mode)
    if mode.startswith(("v12", "v13", "v14")):
        return _build_v12(loop_k, mode)
    """Build the per-core Bass module. loop_k > 1 wraps the whole body in a
    hardware For loop (used only for timing measurements). Modes:
      f16   — v10 pipeline, f16 channels/output; sup/sdn score rows reloaded
              from HBM on the SWDGE ring.
      f16sb — sup/sdn via SBUF->SBUF row-shifted DMA (SWDGE), only the two
              boundary rows come from HBM.
      f16gp — sup/sdn via GpSimd cross-partition tensor_copy.
    """
    from contextlib import ExitStack, nullcontext

    import bass_rust
    import concourse.tile as tile
    from concourse import bacc, mybir
    from concourse.alu_op_type import AluOpType

    f32 = mybir.dt.float32
    f16 = mybir.dt.bfloat16 if mode.startswith("b16") else mybir.dt.float16
    Act = bass_rust.ActivationFunctionType

    nc = bacc.Bacc(None, target_bir_lowering=False)

    v3 = mode.startswith("v3")
    px_dt = f32 if v3 else f16
    xs = nc.dram_tensor("xs", [B_LOCAL, H, W], f32, kind="ExternalInput")
    xr = nc.dram_tensor("xr", [B_LOCAL, C - 1, H, W], f16, kind="ExternalInput")
    pxd = nc.dram_tensor("pxd", [PT, W], px_dt, kind="ExternalInput")
    pyd = nc.dram_tensor("pyd", [NT, PT], f32, kind="ExternalInput")
    ot_dt = f32 if mode in ("f16i", "b16i") else f16
    out = nc.dram_tensor("out", [B_LOCAL, H, W, C], ot_dt, kind="ExternalOutput")

    with tile.TileContext(nc) as tc, ExitStack() as ctx:
        loop = tc.For_i(0, loop_k, 1) if loop_k > 1 else nullcontext()
        ctx.enter_context(loop)
        const = ctx.enter_context(tc.tile_pool(name="const", bufs=1))
        inp = ctx.enter_context(tc.tile_pool(name="inp", bufs=3))
        sp = ctx.enter_context(tc.tile_pool(name="sp", bufs=2))
        mid = ctx.enter_context(tc.tile_pool(name="mid", bufs=2))
        outp = ctx.enter_context(tc.tile_pool(name="outp", bufs=2))

        pxt = const.tile([PT, W], px_dt)
        nc.sync.dma_start(pxt[:], pxd[:])
        pyt = const.tile([PT, NT], f32)
        nc.sync.dma_start(pyt[:], pyd.rearrange("t p -> p t"))
        # px broadcast views: [p][j][w] with j (landmark idx) as a 0-step dim
        pxb = pxt[:].broadcast_to([PT, W, 5]).rearrange("p w j -> p j w")

        def emit_masks_store(b, t, r0, sc, m, m16, ot4, olm, halves):
            for ws in halves:
                n = ws.stop - ws.start
                mh = m16[:, ws]
                nc.vector.tensor_tensor(ot4[:, ws, 0], sc[:, ws], m[:, ws], op=AluOpType.mult)
                nc.vector.tensor_tensor(ot4[:, ws, 1:5], ot4[:, ws, 1:5],
                                        mh.broadcast_to([PT, n, 4]), op=AluOpType.mult)
                mbh = mh.broadcast_to([PT, n, 5]).rearrange("p w j -> p j w")
                oxh = olm[:, ws, :, 0].rearrange("p w j -> p j w")
                oyh = olm[:, ws, :, 1].rearrange("p w j -> p j w")
                nc.vector.tensor_tensor(oxh, oxh, mbh, op=AluOpType.mult)
                nc.vector.tensor_tensor(oyh, oyh, mbh, op=AluOpType.mult)
                nc.sync.dma_start(out[b, r0:r0 + PT, ws, :], ot4[:, ws, :])

        def emit_decode(b, t, r0, v14, sc, m, m16):
            # f16a: no f16 compute on GpSimd (Q7 software emulation of f16 is
            # slow on hw even though the cost model, keyed by op name only,
            # can't see it) — cxp/olx go to DVE instead.
            lm_eng = nc.vector if mode in ("f16a", "f16i", "b16a", "b16i") else nc.gpsimd
            pycol = pyt[:, t:t + 1]
            cxp = mid.tile([PT, W], ot_dt)
            lm_eng.tensor_tensor(cxp[:], v14[:, 0, :], pxt[:], op=AluOpType.add)
            cyp = mid.tile([PT, W], ot_dt)
            nc.scalar.activation(cyp[:], v14[:, 1, :], Act.Identity, bias=pycol, scale=1.0)

            # decode straight into the interleaved output tile, mask in place
            ot = outp.tile([PT, W * C], ot_dt)
            ot4 = ot.rearrange("p (w c) -> p w c", c=C)
            nc.vector.scalar_tensor_tensor(
                ot4[:, :, 1], v14[:, 2, :], -0.5, cxp[:], AluOpType.mult, AluOpType.add)
            nc.vector.scalar_tensor_tensor(
                ot4[:, :, 3], v14[:, 2, :], 0.5, cxp[:], AluOpType.mult, AluOpType.add)
            nc.vector.scalar_tensor_tensor(
                ot4[:, :, 2], v14[:, 3, :], -0.5, cyp[:], AluOpType.mult, AluOpType.add)
            nc.vector.scalar_tensor_tensor(
                ot4[:, :, 4], v14[:, 3, :], 0.5, cyp[:], AluOpType.mult, AluOpType.add)

            # landmarks: channels 5..14 = 5 (x, y) pairs
            lmp = v14[:, 4:C - 1, :].rearrange("p (j k) w -> p j k w", k=2)
            olm = ot4[:, :, 5:C].rearrange("p w (j k) -> p w j k", k=2)
            olx = olm[:, :, :, 0].rearrange("p w j -> p j w")
            oly = olm[:, :, :, 1].rearrange("p w j -> p j w")
            lm_eng.tensor_tensor(olx, lmp[:, :, 0, :], pxb, op=AluOpType.add)
            nc.scalar.activation(oly, lmp[:, :, 1, :], Act.Identity, bias=pycol, scale=1.0)

            # ---- masking ----
            if b == B_LOCAL - 1 and t == NT - 1:
                # last tile: half-width masking so the first half-store
                # overlaps the second half's masks (shrinks the tail)
                emit_masks_store(b, t, r0, sc, m, m16, ot4, olm,
                                 [slice(0, W // 2), slice(W // 2, W)])
                return
            nc.vector.tensor_tensor(ot4[:, :, 0], sc[:], m[:], op=AluOpType.mult)
            mb4 = m16[:].broadcast_to([PT, W, 4])
            nc.vector.tensor_tensor(ot4[:, :, 1:5], ot4[:, :, 1:5], mb4, op=AluOpType.mult)
            mb = m16[:].broadcast_to([PT, W, 5]).rearrange("p w j -> p j w")
            nc.vector.tensor_tensor(olx, olx, mb, op=AluOpType.mult)
            nc.vector.tensor_tensor(oly, oly, mb, op=AluOpType.mult)
            nc.sync.dma_start(out[b, r0:r0 + PT, :, :], ot4[:, :, :])

        for b in range(B_LOCAL):
            for t in range(NT):
                r0 = PT * t

                # DMA ring split: input loads on the ACT HWDGE ring, output
                # store on the SP ring, so the two FIFOs stream in parallel
                # and HBM bandwidth (not one ring) is the binding limit.
                ldq = nc.scalar
                sc = inp.tile([PT, W], f32)
                ldq.dma_start(sc[:], xs[b, r0:r0 + PT, :])
                v14f = inp.tile([PT, (C - 1) * W], f16)
                v14 = v14f.rearrange("p (c w) -> p c w", c=C - 1)
                # split load: deltas+sizes land first so decode starts
                # earlier; landmark channels follow
                ldq.dma_start(v14[:, 0:4, :], xr[b, 0:4, r0:r0 + PT, :].rearrange("c p w -> p c w"))
                ldq.dma_start(v14[:, 4:C - 1, :], xr[b, 4:C - 1, r0:r0 + PT, :].rearrange("c p w -> p c w"))

                # +-1-row shifted score tiles for the vertical max.
                sup = sp.tile([PT, W], f32)
                sdn = sp.tile([PT, W], f32)
                if mode == "f16sb":
                    nc.gpsimd.dma_start(sup[1:PT, :], sc[0:PT - 1, :])
                    rup = max(r0 - 1, 0)
                    nc.gpsimd.dma_start(sup[0:1, :], xs[b, rup:rup + 1, :])
                    nc.gpsimd.dma_start(sdn[0:PT - 1, :], sc[1:PT, :])
                    rdn = min(r0 + PT, H - 1)
                    nc.gpsimd.dma_start(sdn[PT - 1:PT, :], xs[b, rdn:rdn + 1, :])
                elif mode == "f16gp":
                    nc.gpsimd.tensor_copy(sup[1:PT, :], sc[0:PT - 1, :])
                    rup = max(r0 - 1, 0)
                    ldq.dma_start(sup[0:1, :], xs[b, rup:rup + 1, :])
                    nc.gpsimd.tensor_copy(sdn[0:PT - 1, :], sc[1:PT, :])
                    rdn = min(r0 + PT, H - 1)
                    ldq.dma_start(sdn[PT - 1:PT, :], xs[b, rdn:rdn + 1, :])
                else:
                    # HBM reloads on the SWDGE ring; edge rows clamped
                    # (max(a,a,b)==max(a,b) == SAME padding)
                    sq = nc.gpsimd
                    if t > 0:
                        sq.dma_start(sup[:], xs[b, r0 - 1:r0 + PT - 1, :])
                    else:
                        sq.dma_start(sup[0:1, :], xs[b, 0:1, :])
                        sq.dma_start(sup[1:PT, :], xs[b, 0:PT - 1, :])
                    if t < NT - 1:
                        sq.dma_start(sdn[:], xs[b, r0 + 1:r0 + PT + 1, :])
                    else:
                        sq.dma_start(sdn[0:PT - 1, :], xs[b, r0 + 1:H, :])
                        sq.dma_start(sdn[PT - 1:PT, :], xs[b, H - 1:H, :])

                # ---- 3x3 max pool -> peak mask m ----
                # v1 is a rolling scratch: vmax partial, then hmax partial,
                # then the equality mask (WAW deps keep the order correct).
                v1 = mid.tile([PT, W], f32)
                nc.vector.tensor_tensor(v1[:], sup[:], sdn[:], op=AluOpType.max)
                vp = mid.tile([PT, W + 2], f32)
                nc.vector.tensor_tensor(vp[:, 1:W + 1], v1[:], sc[:], op=AluOpType.max)
                # duplicate-edge pad: max(v0,v0,v1) == max(v0,v1) == SAME pooling
                nc.vector.tensor_copy(vp[:, 0:1], vp[:, 1:2])
                nc.vector.tensor_copy(vp[:, W + 1:W + 2], vp[:, W:W + 1])
                nc.vector.tensor_tensor(v1[:], vp[:, 0:W], vp[:, 1:W + 1], op=AluOpType.max)
                pooled = mid.tile([PT, W], f32)
                nc.vector.tensor_tensor(pooled[:], v1[:], vp[:, 2:W + 2], op=AluOpType.max)
                nc.vector.tensor_tensor(v1[:], sc[:], pooled[:], op=AluOpType.is_equal)
                m = mid.tile([PT, W], f32)
                nc.vector.scalar_tensor_tensor(
                    m[:], sc[:], THRESHOLD, v1[:], AluOpType.is_gt, AluOpType.mult)
                if mode in ("f16i", "b16i"):
                    m16 = m  # f32 output path: mask stays f32
                else:
                    m16 = mid.tile([PT, W], f16)
                    nc.scalar.activation(m16[:], m[:], Act.Identity, scale=1.0)

                emit_decode(b, t, r0, v14, sc[:], m, m16)

    nc.compile()
    return nc




def _build_v12(loop_k: int = 1, mode: str = "v12"):
    """v12: planar channel-major output + single fp8 input block.

    Design (vs v7f8): every engine write is flat/contiguous, removing the
    measured strided-16-bit-write penalties entirely; the host transposes
    the [B,H,C,W] bf16 output to [B,H,W,C] f32 during the gather.
      - Input: score plane fp32 (bit-exact peak mask) + ONE fp8(e4m3)
        tensor [B, H, 14, W] holding [dx,dy,sx,sy, lmx*5, lmy*5]; each
        row-tile load is a single contiguous 7168B/partition chunk
        (split 2048B + 5120B so bbox decode starts before landmarks land).
        fp8 on deltas/sizes adds <=~0.4 abs err vs a ~40 budget.
      - Output: [B, H, C, W] bf16; per-partition row = 15360B contiguous.
      - Pooling/mask on DVE f32 (exact); mask written directly as bf16
        (0/1 exact), so all 14 mask multiplies are flat packed bf16 at
        DVE's 2x rate.
      - Engine split per tile: GpSimd ch0 + cx + 5 lmx planes; ACT cy +
        5 lmy planes (bias add); DVE pool (6) + bbox stt (4) + masks (2).
      - sup/sdn shifted score rows: HBM reloads on the SWDGE ring (v12),
        SBUF->SBUF row-shift (v12s).
    Loads on the ACT HWDGE ring, store on the SP ring.
    """
    from contextlib import ExitStack, nullcontext

    import bass_rust
    import concourse.tile as tile
    from concourse import bacc, mybir
    from concourse.alu_op_type import AluOpType

    f32 = mybir.dt.float32
    b16 = mybir.dt.bfloat16
    f8 = mybir.dt.float8e4
    Act = bass_rust.ActivationFunctionType

    nc = bacc.Bacc(None, target_bir_lowering=False)

    CQ = C - 1  # 14 fp8 channels
    xs = nc.dram_tensor("xs", [B_LOCAL, H, W], f32, kind="ExternalInput")
    xq = nc.dram_tensor("xq", [B_LOCAL, H, CQ, W], f8, kind="ExternalInput")
    pxd = nc.dram_tensor("pxd", [PT, W], f32, kind="ExternalInput")
    pyd = nc.dram_tensor("pyd", [NT, PT], f32, kind="ExternalInput")
    out = nc.dram_tensor("out", [B_LOCAL, H, C, W], b16, kind="ExternalOutput")

    parts = mode.split("-")
    mode = parts[0]
    flags = set(parts[1:])
    v13 = mode.startswith(("v13", "v14"))
    v14 = mode.startswith("v14")
    sbuf_shift = mode == "v12s"
    pe_shift = mode in ("v12m", "v12pe") or v13
    pe_bbox = mode == "v12pe" or v13
    lm_pe = mode == "v13b" or v14
    hmax_gp = mode == "v14p"  # may also be set via the -hgp flag below
    if pe_bbox:
        # bbox decode on the tensor engine: x1 = I@dx + (-0.5 I)@sx + ones(x)px
        # (fp8 stationaries exact on 0/1/±0.5; px f16 exact for ints <= 2046;
        # py is folded in as the ACT bias during the PSUM->bf16 conversion)
        sid8 = nc.dram_tensor("sid8", [PT, PT], f8, kind="ExternalInput")
        smh8 = nc.dram_tensor("smh8", [PT, PT], f8, kind="ExternalInput")
        sph8 = nc.dram_tensor("sph8", [PT, PT], f8, kind="ExternalInput")
        sone16 = nc.dram_tensor("sone16", [1, PT], mybir.dt.float16, kind="ExternalInput")
        px16d = nc.dram_tensor("px16d", [1, W], mybir.dt.float16, kind="ExternalInput")
    if pe_shift:
        # superdiagonal / subdiagonal 0-1 shift matrices WITHOUT corner
        # clamps: the boundary output row of S.T@sc is exactly 0, and a
        # second K=1 matmul accumulates e_row (x) boundary_row into the
        # same PSUM bank (0 + x is exact; 0/1 fp32 products are exact)
        smu = nc.dram_tensor("smu", [PT, PT], f32, kind="ExternalInput")
        smd = nc.dram_tensor("smd", [PT, PT], f32, kind="ExternalInput")
        se127 = nc.dram_tensor("se127", [1, PT], f32, kind="ExternalInput")

    # (sc > 0.5) & (sc == pooled)  ==  max(pooled, nextafter(0.5)) <= sc
    # (pooled >= sc always, and sc >= nextafter(0.5) iff sc > 0.5 in f32)
    THR_NEXT = float(np.nextafter(np.float32(THRESHOLD), np.float32(1.0)))

    with tile.TileContext(nc) as tc, ExitStack() as ctx:
        loop = tc.For_i(0, loop_k, 1) if loop_k > 1 else nullcontext()
        ctx.enter_context(loop)
        const = ctx.enter_context(tc.tile_pool(name="const", bufs=1))
        inp = ctx.enter_context(tc.tile_pool(name="inp", bufs=5 if "in5" in flags else 4))
        mid = ctx.enter_context(tc.tile_pool(name="mid", bufs=3))
        outp = ctx.enter_context(tc.tile_pool(name="outp", bufs=4 if "ob4" in flags else 3))
        if pe_shift:
            psum = ctx.enter_context(tc.tile_pool(
                name="psum", bufs=1 if lm_pe else 2, space="PSUM"))
            sp = None
        else:
            sp = ctx.enter_context(tc.tile_pool(name="sp", bufs=2))

        pxt = const.tile([PT, W], f32)
        nc.sync.dma_start(pxt[:], pxd[:])
        pyt = const.tile([PT, NT], f32)
        nc.sync.dma_start(pyt[:], pyd.rearrange("t p -> p t"))
        pxb = pxt[:].broadcast_to([PT, W, 5]).rearrange("p w j -> p j w")
        if pe_shift:
            smu_t = const.tile([PT, PT], f32)
            nc.sync.dma_start(smu_t[:], smu[:])
            smd_t = const.tile([PT, PT], f32)
            nc.sync.dma_start(smd_t[:], smd[:])
            se127_t = const.tile([1, PT], f32)
            nc.sync.dma_start(se127_t[:], se127[:])
        if pe_bbox:
            f16 = mybir.dt.float16
            sid8_t = const.tile([PT, PT], f8)
            nc.sync.dma_start(sid8_t[:], sid8[:])
            smh8_t = const.tile([PT, PT], f8)
            nc.sync.dma_start(smh8_t[:], smh8[:])
            sph8_t = const.tile([PT, PT], f8)
            nc.sync.dma_start(sph8_t[:], sph8[:])
            sone16_t = const.tile([1, PT], f16)
            nc.sync.dma_start(sone16_t[:], sone16[:])
            px16_t = const.tile([1, W], f16)
            nc.sync.dma_start(px16_t[:], px16d[:])
            bpsum = ctx.enter_context(
                tc.tile_pool(name="bpsum", bufs=1, space="PSUM"))
            if lm_pe:
                lpsum = ctx.enter_context(
                    tc.tile_pool(name="lpsum", bufs=1, space="PSUM"))

        reps = 2 if "dual" in flags else 1
        for rep in range(reps):
          for b in range(B_LOCAL):
            for t in range(NT):
                r0 = PT * t
                pycol = pyt[:, t:t + 1]
                last = b == B_LOCAL - 1 and t == NT - 1

                # ---- loads on the ACT HWDGE ring ----
                ldq = nc.scalar
                stq = nc.sync
                sc = inp.tile([PT, W], f32)
                ldq.dma_start(sc[:], xs[b, r0:r0 + PT, :])
                qf = inp.tile([PT, CQ * W], f8)
                qv = qf.rearrange("p (c w) -> p c w", c=CQ)
                ldq.dma_start(qv[:, 0:4, :], xq[b, r0:r0 + PT, 0:4, :])
                if "xq" not in flags:
                    ldq.dma_start(qv[:, 4:CQ, :], xq[b, r0:r0 + PT, 4:CQ, :])
                elif "xlmx" not in flags and "xcomp" not in flags:
                    # keep the lm channels allocated for the decode reads
                    ldq.dma_start(qv[:, 4:CQ, 0:1], xq[b, r0:r0 + PT, 4:CQ, 0:1])

                # ---- sup/sdn shifted score rows + vertical max ----
                v1 = mid.tile([PT, W], f32)
                vp = mid.tile([PT, W + 2], f32)
                if "xcomp" in flags:
                    ob = outp.tile([PT, C * W], b16)
                    obv = ob.rearrange("p (c w) -> p c w", c=C)
                    # touch ob once so the store has a producer
                    nc.vector.tensor_copy(obv[:, 0, :], sc[:])
                    stq.dma_start(out[b, r0:r0 + PT, :, :], obv[:, :, :])
                    continue
                if pe_shift:
                    # Shift matrices leave boundary rows exactly 0 (a 0 can
                    # never flip the mask: mask needs sc > 0.5 > 0). The true
                    # up-boundary row lands via a 1-partition DVE max (row 0
                    # is a legal start partition); the down-boundary row (row
                    # 127 - illegal DVE start) accumulates via a K=1 matmul.
                    uprow = mid.tile([1, W], f32)
                    rup = max(r0 - 1, 0)
                    ldq.dma_start(uprow[:], xs[b, rup:rup + 1, :])
                    dnrow = mid.tile([1, W], f32)
                    rdn = min(r0 + PT, H - 1)
                    ldq.dma_start(dnrow[:], xs[b, rdn:rdn + 1, :])
                    sup_ps = psum.tile([PT, W], f32)
                    nc.tensor.matmul(sup_ps[:], smu_t[:], sc[:], start=True, stop=True)
                    sdn_ps = psum.tile([PT, W], f32)
                    nc.tensor.matmul(sdn_ps[:], smd_t[:], sc[:], start=True, stop=False)
                    nc.tensor.matmul(sdn_ps[:], se127_t[:], dnrow[:], start=False, stop=True)
                    nc.vector.tensor_tensor(v1[:], sup_ps[:], sc[:], op=AluOpType.max)
                    nc.vector.tensor_tensor(v1[0:1, :], v1[0:1, :], uprow[:],
                                            op=AluOpType.max)
                    nc.vector.tensor_tensor(vp[:, 1:W + 1], v1[:], sdn_ps[:],
                                            op=AluOpType.max)
                else:
                    sup = sp.tile([PT, W], f32)
                    sdn = sp.tile([PT, W], f32)
                    sq = nc.gpsimd
                    if sbuf_shift:
                        sq.dma_start(sup[1:PT, :], sc[0:PT - 1, :])
                        rup = max(r0 - 1, 0)
                        sq.dma_start(sup[0:1, :], xs[b, rup:rup + 1, :])
                        sq.dma_start(sdn[0:PT - 1, :], sc[1:PT, :])
                        rdn = min(r0 + PT, H - 1)
                        sq.dma_start(sdn[PT - 1:PT, :], xs[b, rdn:rdn + 1, :])
                    else:
                        # HBM reloads; edge rows clamped (max(a,a,b)==max(a,b))
                        if t > 0:
                            sq.dma_start(sup[:], xs[b, r0 - 1:r0 + PT - 1, :])
                        else:
                            sq.dma_start(sup[0:1, :], xs[b, 0:1, :])
                            sq.dma_start(sup[1:PT, :], xs[b, 0:PT - 1, :])
                        if t < NT - 1:
                            sq.dma_start(sdn[:], xs[b, r0 + 1:r0 + PT + 1, :])
                        else:
                            sq.dma_start(sdn[0:PT - 1, :], xs[b, r0 + 1:H, :])
                            sq.dma_start(sdn[PT - 1:PT, :], xs[b, H - 1:H, :])
                    nc.vector.tensor_tensor(v1[:], sup[:], sdn[:], op=AluOpType.max)
                    nc.vector.tensor_tensor(vp[:, 1:W + 1], v1[:], sc[:],
                                            op=AluOpType.max)

                # ---- horizontal max + peak mask (f32, exact) ----
                heng = nc.gpsimd if (hmax_gp or "hgp" in flags) else nc.vector
                nc.vector.tensor_copy(vp[:, 0:1], vp[:, 1:2])
                nc.vector.tensor_copy(vp[:, W + 1:W + 2], vp[:, W:W + 1])
                heng.tensor_tensor(v1[:], vp[:, 0:W], vp[:, 1:W + 1], op=AluOpType.max)
                pooled = mid.tile([PT, W], f32)
                heng.tensor_tensor(pooled[:], v1[:], vp[:, 2:W + 2], op=AluOpType.max)
                # mask directly in bf16 (0/1 exact) -> all mask mults run
                # as flat packed bf16
                m16 = mid.tile([PT, W], b16)
                meng = nc.gpsimd if "mgp" in flags else nc.vector
                meng.scalar_tensor_tensor(
                    m16[:], pooled[:], THR_NEXT, sc[:], AluOpType.max, AluOpType.is_le)

                # ---- decode into planar bf16 output planes ----
                ob = outp.tile([PT, C * W], b16)
                obv = ob.rearrange("p (c w) -> p c w", c=C)

                if pe_bbox:
                    # x1/x2 = I@dx + (-+0.5 I)@sx + ones(x)px ; y1/y2 = I@dy
                    # + (-+0.5 I)@sy, py added as ACT bias at the conversion
                    psx = bpsum.tile([PT, 2 * W], f32)
                    psx2 = psx.rearrange("p (j w) -> p j w", j=2)
                    psy = bpsum.tile([PT, 2 * W], f32)
                    psy2 = psy.rearrange("p (j w) -> p j w", j=2)
                    nc.tensor.matmul(psx2[:, 0, :], sid8_t[:], qv[:, 0, :], start=True, stop=False)
                    nc.tensor.matmul(psx2[:, 0, :], smh8_t[:], qv[:, 2, :], start=False, stop=False)
                    nc.tensor.matmul(psx2[:, 0, :], sone16_t[:], px16_t[:], start=False, stop=True)
                    nc.tensor.matmul(psx2[:, 1, :], sid8_t[:], qv[:, 0, :], start=True, stop=False)
                    nc.tensor.matmul(psx2[:, 1, :], sph8_t[:], qv[:, 2, :], start=False, stop=False)
                    nc.tensor.matmul(psx2[:, 1, :], sone16_t[:], px16_t[:], start=False, stop=True)
                    nc.tensor.matmul(psy2[:, 0, :], sid8_t[:], qv[:, 1, :], start=True, stop=False)
                    nc.tensor.matmul(psy2[:, 0, :], smh8_t[:], qv[:, 3, :], start=False, stop=True)
                    nc.tensor.matmul(psy2[:, 1, :], sid8_t[:], qv[:, 1, :], start=True, stop=False)
                    nc.tensor.matmul(psy2[:, 1, :], sph8_t[:], qv[:, 3, :], start=False, stop=True)
                    nc.scalar.activation(obv[:, 1:5:2, :], psx2[:, :, :], Act.Copy)
                    nc.scalar.activation(obv[:, 2:5:2, :], psy2[:, :, :], Act.Identity,
                                         bias=pycol, scale=1.0)
                else:
                    cx = mid.tile([PT, W], f32)
                    nc.gpsimd.tensor_tensor(cx[:], qv[:, 0, :], pxt[:], op=AluOpType.add)
                    cy = mid.tile([PT, W], f32)
                    nc.scalar.activation(cy[:], qv[:, 1, :], Act.Identity, bias=pycol, scale=1.0)

                    nc.vector.scalar_tensor_tensor(
                        obv[:, 1, :], qv[:, 2, :], -0.5, cx[:], AluOpType.mult, AluOpType.add)
                    nc.vector.scalar_tensor_tensor(
                        obv[:, 2, :], qv[:, 3, :], -0.5, cy[:], AluOpType.mult, AluOpType.add)
                    nc.vector.scalar_tensor_tensor(
                        obv[:, 3, :], qv[:, 2, :], 0.5, cx[:], AluOpType.mult, AluOpType.add)
                    nc.vector.scalar_tensor_tensor(
                        obv[:, 4, :], qv[:, 3, :], 0.5, cy[:], AluOpType.mult, AluOpType.add)

                if v13:
                    # score channel: unmasked bf16 copy; masked by the 0:5
                    # mask op below
                    if "cgp" in flags:
                        nc.gpsimd.tensor_copy(obv[:, 0, :], sc[:])
                    else:
                        nc.scalar.activation(obv[:, 0, :], sc[:], Act.Copy)
                if lm_pe:
                    n_pe_lm = 4 if v14 else 2
                    for w0 in range(0, n_pe_lm, 2):
                        psl = lpsum.tile([PT, 2 * W], f32)
                        psl2 = psl.rearrange("p (j w) -> p j w", j=2)
                        for j in range(2):
                            nc.tensor.matmul(psl2[:, j, :], sid8_t[:],
                                             qv[:, 4 + w0 + j, :],
                                             start=True, stop=False)
                            nc.tensor.matmul(psl2[:, j, :], sone16_t[:], px16_t[:],
                                             start=False, stop=True)
                        c0 = 5 + 2 * w0
                        nc.scalar.activation(obv[:, c0:c0 + 4:2, :], psl2[:, :, :],
                                             Act.Copy)
                    ngp = 5 - n_pe_lm
                    cgp = 5 + 2 * n_pe_lm
                    if "lmd" in flags and ngp > 1:
                        # last x-plane on DVE: shortens the GpSimd pole that
                        # gates the final mask+store chunk
                        nc.gpsimd.tensor_tensor(
                            obv[:, cgp:C - 2:2, :], qv[:, 4 + n_pe_lm:8, :],
                            pxt[:].broadcast_to([PT, W, ngp - 1]).rearrange("p w j -> p j w"),
                            op=AluOpType.add)
                        nc.vector.tensor_tensor(obv[:, C - 2, :], qv[:, 8, :],
                                                pxt[:], op=AluOpType.add)
                    else:
                        nc.gpsimd.tensor_tensor(
                            obv[:, cgp:C:2, :], qv[:, 4 + n_pe_lm:9, :],
                            pxt[:].broadcast_to([PT, W, ngp]).rearrange("p w j -> p j w"),
                            op=AluOpType.add)
                elif "xlmx" not in flags:
                    nc.gpsimd.tensor_tensor(obv[:, 5:C:2, :], qv[:, 4:9, :], pxb,
                                            op=AluOpType.add)
                nc.scalar.activation(obv[:, 6:C:2, :], qv[:, 9:CQ, :], Act.Identity,
                                     bias=pycol, scale=1.0)

                # ---- score channel + masks + store ----
                mb4 = m16[:].broadcast_to([PT, W, 4]).rearrange("p w j -> p j w")
                mb5 = m16[:].broadcast_to([PT, W, 5]).rearrange("p w j -> p j w")
                mb10 = m16[:].broadcast_to([PT, W, 10]).rearrange("p w j -> p j w")
                mb14 = m16[:].broadcast_to([PT, W, 14]).rearrange("p w j -> p j w")
                if v13:
                    # split mask+store: the 0:5 half (score+bbox) never waits
                    # for the landmark decodes; its store overlaps them
                    nc.vector.tensor_tensor(obv[:, 0:5, :], obv[:, 0:5, :], mb5,
                                            op=AluOpType.mult)
                    if "xst" not in flags:
                        stq.dma_start(out[b, r0:r0 + PT, 0:5, :], obv[:, 0:5, :])
                    if "tri" in flags:
                        # 5:9 (PE-decoded lm planes + lmy) masks/stores while
                        # GpSimd still decodes the last x-planes (9:15)
                        mb4b = m16[:].broadcast_to([PT, W, 4]).rearrange("p w j -> p j w")
                        mb6 = m16[:].broadcast_to([PT, W, 6]).rearrange("p w j -> p j w")
                        nc.vector.tensor_tensor(obv[:, 5:9, :], obv[:, 5:9, :], mb4b,
                                                op=AluOpType.mult)
                        stq.dma_start(out[b, r0:r0 + PT, 5:9, :], obv[:, 5:9, :])
                        nc.vector.tensor_tensor(obv[:, 9:C, :], obv[:, 9:C, :], mb6,
                                                op=AluOpType.mult)
                        stq.dma_start(out[b, r0:r0 + PT, 9:C, :], obv[:, 9:C, :])
                    else:
                        nc.vector.tensor_tensor(obv[:, 5:C, :], obv[:, 5:C, :], mb10,
                                                op=AluOpType.mult)
                        if "xst" not in flags:
                            stq.dma_start(out[b, r0:r0 + PT, 5:C, :], obv[:, 5:C, :])
                else:
                    nc.gpsimd.tensor_tensor(obv[:, 0, :], sc[:], m16[:], op=AluOpType.mult)
                    if last and not flags:
                        # tail shrink: mask+store 5 bbox planes first so that
                        # store overlaps the landmark masks
                        nc.vector.tensor_tensor(obv[:, 1:5, :], obv[:, 1:5, :], mb4,
                                                op=AluOpType.mult)
                        stq.dma_start(out[b, r0:r0 + PT, 0:5, :], obv[:, 0:5, :])
                        nc.vector.tensor_tensor(obv[:, 5:C, :], obv[:, 5:C, :], mb10,
                                                op=AluOpType.mult)
                        stq.dma_start(out[b, r0:r0 + PT, 5:C, :], obv[:, 5:C, :])
                    else:
                        if "xmask" not in flags:
                            nc.vector.tensor_tensor(obv[:, 1:C, :], obv[:, 1:C, :], mb14,
                                                    op=AluOpType.mult)
                        if "xst" not in flags:
                            if "st2" in flags:
                                stq.dma_start(out[b, r0:r0 + PT, 0:12, :], obv[:, 0:12, :])
                                ldq.dma_start(out[b, r0:r0 + PT, 12:C, :], obv[:, 12:C, :])
                            else:
                                stq.dma_start(out[b, r0:r0 + PT, :, :], obv[:, :, :])

    nc.compile()
    return nc


def _build_v4(loop_k: int = 1, mode: str = "v4"):
    """v4: bf16-I/O kernel tuned to measured TRN2 rates.

    Measured hw rules this design follows:
      - 16-bit strided SBUF writes on DVE/ACT are ~4x slow -> never emitted
        (the one unavoidable one, the score channel, goes to GpSimd, which
        is software and stride-agnostic).
      - packed bf16 DVE ops run 2x; f32->bf16 packed-out ops ~1x.
      - bf16 reads are free on every engine.
    Structure per tile:
      bbox channels: all-packed bf16 DVE chain on pair-interleaved inputs
        (cen2/size2 premasked, so the two stt ops emit masked bf16 pairs
        straight into the interleaved output tile).
      landmarks: planar bf16 loads; x+px on GpSimd, y+py on ACT, assembled
        f32 into an interleaved scratch; one packed-10 DVE mult by the f32
        mask converts+masks into the output tile.
      score: GpSimd writes sc*m into the strided channel-0 lane.
      pooling/mask: DVE f32 (exact), optional offload knobs -> GpSimd.
    Modes: v4 (sup/sdn HBM reload), v4s (SBUF->SBUF row-shift on SWDGE),
      v4p (v4s + eq/vmax offloaded to GpSimd).
    """
    from contextlib import ExitStack, nullcontext

    import bass_rust
    import concourse.tile as tile
    from concourse import bacc, mybir
    from concourse.alu_op_type import AluOpType

    f32 = mybir.dt.float32
    b16 = mybir.dt.bfloat16
    Act = bass_rust.ActivationFunctionType

    nc = bacc.Bacc(None, target_bir_lowering=False)

    xs = nc.dram_tensor("xs", [B_LOCAL, H, W], f32, kind="ExternalInput")
    xp2 = nc.dram_tensor("xp2", [B_LOCAL, 2, H, 2 * W], b16, kind="ExternalInput")
    xl = nc.dram_tensor("xl", [B_LOCAL, 10, H, W], b16, kind="ExternalInput")
    pxd = nc.dram_tensor("pxd", [PT, W], f32, kind="ExternalInput")
    pyd = nc.dram_tensor("pyd", [NT, PT], f32, kind="ExternalInput")
    px2d = nc.dram_tensor("px2d", [PT, 2 * W], b16, kind="ExternalInput")
    sy2d = nc.dram_tensor("sy2d", [PT, 2 * W], b16, kind="ExternalInput")
    out = nc.dram_tensor("out", [B_LOCAL, H, W, C], b16, kind="ExternalOutput")

    with tile.TileContext(nc) as tc, ExitStack() as ctx:
        loop = tc.For_i(0, loop_k, 1) if loop_k > 1 else nullcontext()
        ctx.enter_context(loop)
        const = ctx.enter_context(tc.tile_pool(name="const", bufs=1))
        inp = ctx.enter_context(tc.tile_pool(name="inp", bufs=3))
        sp = ctx.enter_context(tc.tile_pool(name="sp", bufs=2))
        mid = ctx.enter_context(tc.tile_pool(name="mid", bufs=2))
        outp = ctx.enter_context(tc.tile_pool(name="outp", bufs=2))

        pxt = const.tile([PT, W], f32)
        nc.sync.dma_start(pxt[:], pxd[:])
        pyt = const.tile([PT, NT], f32)
        nc.sync.dma_start(pyt[:], pyd.rearrange("t p -> p t"))
        px2 = const.tile([PT, 2 * W], b16)
        nc.sync.dma_start(px2[:], px2d[:])
        sy2 = const.tile([PT, 2 * W], b16)
        nc.sync.dma_start(sy2[:], sy2d[:])
        pxb = pxt[:].broadcast_to([PT, W, 5]).rearrange("p w j -> p j w")

        sbuf_shift = mode in ("v4s", "v4p")
        gp_pool = mode == "v4p"

        reps = 2 if "dual" in flags else 1
        for rep in range(reps):
          for b in range(B_LOCAL):
            for t in range(NT):
                r0 = PT * t
                pycol = pyt[:, t:t + 1]
                last = b == B_LOCAL - 1 and t == NT - 1

                # ---- loads: SP HWDGE ring ----
                sc = inp.tile([PT, W], f32)
                nc.sync.dma_start(sc[:], xs[b, r0:r0 + PT, :])
                p2 = inp.tile([PT, 2 * 2 * W], b16)
                p2v = p2.rearrange("p (c w) -> p c w", c=2)
                nc.sync.dma_start(p2v[:, :, :], xp2[b, :, r0:r0 + PT, :].rearrange("c p w -> p c w"))
                l10 = inp.tile([PT, 10 * W], b16)
                l10v = l10.rearrange("p (c w) -> p c w", c=10)
                nc.sync.dma_start(l10v[:, :, :], xl[b, :, r0:r0 + PT, :].rearrange("c p w -> p c w"))

                # ---- sup/sdn shifted score rows ----
                sup = sp.tile([PT, W], f32)
                sdn = sp.tile([PT, W], f32)
                if sbuf_shift:
                    nc.gpsimd.dma_start(sup[1:PT, :], sc[0:PT - 1, :])
                    rup = max(r0 - 1, 0)
                    nc.gpsimd.dma_start(sup[0:1, :], xs[b, rup:rup + 1, :])
                    nc.gpsimd.dma_start(sdn[0:PT - 1, :], sc[1:PT, :])
                    rdn = min(r0 + PT, H - 1)
                    nc.gpsimd.dma_start(sdn[PT - 1:PT, :], xs[b, rdn:rdn + 1, :])
                else:
                    sq = nc.gpsimd
                    if t > 0:
                        sq.dma_start(sup[:], xs[b, r0 - 1:r0 + PT - 1, :])
                    else:
                        sq.dma_start(sup[0:1, :], xs[b, 0:1, :])
                        sq.dma_start(sup[1:PT, :], xs[b, 0:PT - 1, :])
                    if t < NT - 1:
                        sq.dma_start(sdn[:], xs[b, r0 + 1:r0 + PT + 1, :])
                    else:
                        sq.dma_start(sdn[0:PT - 1, :], xs[b, r0 + 1:H, :])
                        sq.dma_start(sdn[PT - 1:PT, :], xs[b, H - 1:H, :])

                # ---- 3x3 max pool -> peak mask m (f32, exact) ----
                v1 = mid.tile([PT, W], f32)
                veng = nc.gpsimd if gp_pool else nc.vector
                veng.tensor_tensor(v1[:], sup[:], sdn[:], op=AluOpType.max)
                vp = mid.tile([PT, W + 2], f32)
                nc.vector.tensor_tensor(vp[:, 1:W + 1], v1[:], sc[:], op=AluOpType.max)
                nc.vector.tensor_copy(vp[:, 0:1], vp[:, 1:2])
                nc.vector.tensor_copy(vp[:, W + 1:W + 2], vp[:, W:W + 1])
                nc.vector.tensor_tensor(v1[:], vp[:, 0:W], vp[:, 1:W + 1], op=AluOpType.max)
                pooled = mid.tile([PT, W], f32)
                nc.vector.tensor_tensor(pooled[:], v1[:], vp[:, 2:W + 2], op=AluOpType.max)
                eeng = nc.gpsimd if gp_pool else nc.vector
                eeng.tensor_tensor(v1[:], sc[:], pooled[:], op=AluOpType.is_equal)
                m = mid.tile([PT, W], f32)
                nc.vector.scalar_tensor_tensor(
                    m[:], sc[:], THRESHOLD, v1[:], AluOpType.is_gt, AluOpType.mult)
                # pair-duplicated bf16 mask (ACT: bcast-in, packed-out)
                m2 = mid.tile([PT, 2 * W], b16)
                nc.scalar.activation(m2.rearrange("p (w j) -> p w j", j=2),
                                     m[:].broadcast_to([PT, W, 2]),
                                     Act.Identity, scale=1.0)

                ot = outp.tile([PT, W * C], b16)
                ot4 = ot.rearrange("p (w c) -> p w c", c=C)

                # ---- bbox: all-packed bf16 DVE chain (2x) ----
                c1 = mid.tile([PT, 2 * W], b16)
                nc.vector.tensor_tensor(c1[:], p2v[:, 0, :], px2[:], op=AluOpType.add)
                c2 = mid.tile([PT, 2 * W], b16)
                nc.vector.scalar_tensor_tensor(
                    c2[:], sy2[:], pycol, c1[:], AluOpType.mult, AluOpType.add)
                cen2m = mid.tile([PT, 2 * W], b16)
                nc.vector.tensor_tensor(cen2m[:], c2[:], m2[:], op=AluOpType.mult)
                szm2 = mid.tile([PT, 2 * W], b16)
                nc.vector.tensor_tensor(szm2[:], p2v[:, 1, :], m2[:], op=AluOpType.mult)
                c2p = cen2m.rearrange("p (w j) -> p w j", j=2)
                s2p = szm2.rearrange("p (w j) -> p w j", j=2)
                nc.vector.scalar_tensor_tensor(
                    ot4[:, :, 1:3], s2p, -0.5, c2p, AluOpType.mult, AluOpType.add)
                nc.vector.scalar_tensor_tensor(
                    ot4[:, :, 3:5], s2p, 0.5, c2p, AluOpType.mult, AluOpType.add)

                # ---- score channel: GpSimd (software, stride-agnostic) ----
                nc.gpsimd.tensor_tensor(ot4[:, :, 0], sc[:], m[:], op=AluOpType.mult)

                # ---- landmarks: f32 interleaved scratch, mask+convert on DVE ----
                lm32 = mid.tile([PT, W * 10], f32)
                lmi = lm32.rearrange("p (w j) -> p w j", j=10)
                olx = lmi[:, :, 0:10:2].rearrange("p w j -> p j w")
                oly = lmi[:, :, 1:10:2].rearrange("p w j -> p j w")
                nc.gpsimd.tensor_tensor(olx, l10v[:, 0:5, :], pxb, op=AluOpType.add)
                nc.scalar.activation(oly, l10v[:, 5:10, :], Act.Identity,
                                     bias=pycol, scale=1.0)
                if last:
                    # tail shrink: half-width mask+store so the first half's
                    # store overlaps the second half's masking
                    for ws in (slice(0, W // 2), slice(W // 2, W)):
                        n = ws.stop - ws.start
                        nc.vector.tensor_tensor(
                            ot4[:, ws, 5:15], lmi[:, ws, :],
                            m[:, ws].broadcast_to([PT, n, 10]), op=AluOpType.mult)
                        stq.dma_start(out[b, r0:r0 + PT, ws, :], ot4[:, ws, :])
                else:
                    nc.vector.tensor_tensor(
                        ot4[:, :, 5:15], lmi[:, :, :],
                        m[:].broadcast_to([PT, W, 10]), op=AluOpType.mult)
                    # ---- store on the ACT HWDGE ring ----
                    nc.scalar.dma_start(out[b, r0:r0 + PT, :, :], ot4[:, :, :])

    nc.compile()
    return nc




def _build_v6(loop_k: int = 1, mode: str = "v6"):
    """v6: v10's proven f32-assembly structure + bf16 I/O, tuned to measured
    TRN2 rates. bf16 reads are free on all engines; 16-bit strided writes are
    ~4x slow so every 16-bit write is last-dim-contiguous (the masks double as
    the f32->bf16 conversion); the score channel's unavoidable strided bf16
    write goes to GpSimd (software, stride-agnostic).
    Per tile: pooling+mask on DVE f32 (exact); bbox stt decode into an
    interleaved [p][w][4] f32 scratch (strided f32 writes ~1.7x, acceptable);
    landmarks x+px on GpSimd / y+py on ACT into [p][w][10] f32 scratch;
    masks: ch1:5 and ch5:15 packed-out f32->bf16 multiplies on DVE, ch0 on
    GpSimd. Store on the ACT HWDGE ring, loads on SP, sup/sdn on SWDGE.
    """
    from contextlib import ExitStack, nullcontext

    import bass_rust
    import concourse.tile as tile
    from concourse import bacc, mybir
    from concourse.alu_op_type import AluOpType

    f32 = mybir.dt.float32
    b16 = mybir.dt.bfloat16
    Act = bass_rust.ActivationFunctionType

    nc = bacc.Bacc(None, target_bir_lowering=False)

    # v7*: v10's DMA queue layout — loads on the ACT HWDGE ring, stores on
    # the (otherwise empty) SP ring, so a store waiting on the masks never
    # blocks the next tile's ACT compute at the queue head.
    v7 = mode.startswith(("v7", "v8", "v9"))
    fp8 = mode in ("v6f8", "v7f8") or mode.startswith(("v8", "v9"))
    v8 = mode.startswith("v8")
    # v8a: inp bufs=4; v8b: +sup/sdn on the SP HWDGE ring; v8c: +mid bufs=3
    # v9a: mid bufs=3 (inp stays 3); v9c: cxp on DVE instead of GpSimd
    sup_sp = mode in ("v8b", "v8c")
    in_bufs = 4 if v8 else 3
    mid_bufs = 3 if mode in ("v8c", "v9a") else 2
    cxp_eng = "dve" if mode == "v9c" else "gp"
    f8 = mybir.dt.float8e4
    xs = nc.dram_tensor("xs", [B_LOCAL, H, W], f32, kind="ExternalInput")
    nch = 4 if fp8 else C - 1
    xr = nc.dram_tensor("xr", [B_LOCAL, nch, H, W], b16, kind="ExternalInput")
    if fp8:
        xl8 = nc.dram_tensor("xl8", [B_LOCAL, 10, H, W], f8, kind="ExternalInput")
    pxd = nc.dram_tensor("pxd", [PT, W], f32, kind="ExternalInput")
    pyd = nc.dram_tensor("pyd", [NT, PT], f32, kind="ExternalInput")
    out = nc.dram_tensor("out", [B_LOCAL, H, W, C], b16, kind="ExternalOutput")

    with tile.TileContext(nc) as tc, ExitStack() as ctx:
        loop = tc.For_i(0, loop_k, 1) if loop_k > 1 else nullcontext()
        ctx.enter_context(loop)
        const = ctx.enter_context(tc.tile_pool(name="const", bufs=1))
        inp = ctx.enter_context(tc.tile_pool(name="inp", bufs=in_bufs))
        sp = ctx.enter_context(tc.tile_pool(name="sp", bufs=2))
        mid = ctx.enter_context(tc.tile_pool(name="mid", bufs=mid_bufs))
        scr = ctx.enter_context(tc.tile_pool(name="scr", bufs=2))
        outp = ctx.enter_context(tc.tile_pool(name="outp", bufs=2))

        pxt = const.tile([PT, W], f32)
        nc.sync.dma_start(pxt[:], pxd[:])
        pyt = const.tile([PT, NT], f32)
        nc.sync.dma_start(pyt[:], pyd.rearrange("t p -> p t"))
        pxb = pxt[:].broadcast_to([PT, W, 5]).rearrange("p w j -> p j w")

        reps = 2 if "dual" in flags else 1
        for rep in range(reps):
          for b in range(B_LOCAL):
            for t in range(NT):
                r0 = PT * t
                pycol = pyt[:, t:t + 1]
                last = b == B_LOCAL - 1 and t == NT - 1

                # ---- loads on the SP HWDGE ring ----
                ldq = nc.scalar if v7 else nc.sync
                stq = nc.sync if v7 else nc.scalar
                sc = inp.tile([PT, W], f32)
                ldq.dma_start(sc[:], xs[b, r0:r0 + PT, :])
                v14f = inp.tile([PT, nch * W], b16)
                v14 = v14f.rearrange("p (c w) -> p c w", c=nch)
                ldq.dma_start(v14[:, 0:4, :], xr[b, 0:4, r0:r0 + PT, :].rearrange("c p w -> p c w"))
                if fp8:
                    l8f = inp.tile([PT, 10 * W], f8)
                    l8 = l8f.rearrange("p (c w) -> p c w", c=10)
                    ldq.dma_start(l8[:, :, :], xl8[b, :, r0:r0 + PT, :].rearrange("c p w -> p c w"))
                else:
                    ldq.dma_start(v14[:, 4:C - 1, :], xr[b, 4:C - 1, r0:r0 + PT, :].rearrange("c p w -> p c w"))

                # ---- sup/sdn: HBM reloads on the SWDGE ring ----
                sup = sp.tile([PT, W], f32)
                sdn = sp.tile([PT, W], f32)
                sq = nc.sync if sup_sp else nc.gpsimd
                if t > 0:
                    sq.dma_start(sup[:], xs[b, r0 - 1:r0 + PT - 1, :])
                else:
                    sq.dma_start(sup[0:1, :], xs[b, 0:1, :])
                    sq.dma_start(sup[1:PT, :], xs[b, 0:PT - 1, :])
                if t < NT - 1:
                    sq.dma_start(sdn[:], xs[b, r0 + 1:r0 + PT + 1, :])
                else:
                    sq.dma_start(sdn[0:PT - 1, :], xs[b, r0 + 1:H, :])
                    sq.dma_start(sdn[PT - 1:PT, :], xs[b, H - 1:H, :])

                # ---- 3x3 max pool -> peak mask m (f32, exact) ----
                v1 = mid.tile([PT, W], f32)
                nc.vector.tensor_tensor(v1[:], sup[:], sdn[:], op=AluOpType.max)
                vp = mid.tile([PT, W + 2], f32)
                nc.vector.tensor_tensor(vp[:, 1:W + 1], v1[:], sc[:], op=AluOpType.max)
                nc.vector.tensor_copy(vp[:, 0:1], vp[:, 1:2])
                nc.vector.tensor_copy(vp[:, W + 1:W + 2], vp[:, W:W + 1])
                nc.vector.tensor_tensor(v1[:], vp[:, 0:W], vp[:, 1:W + 1], op=AluOpType.max)
                pooled = mid.tile([PT, W], f32)
                nc.vector.tensor_tensor(pooled[:], v1[:], vp[:, 2:W + 2], op=AluOpType.max)
                nc.vector.tensor_tensor(v1[:], sc[:], pooled[:], op=AluOpType.is_equal)
                m = mid.tile([PT, W], f32)
                nc.vector.scalar_tensor_tensor(
                    m[:], sc[:], THRESHOLD, v1[:], AluOpType.is_gt, AluOpType.mult)

                # ---- decode ----
                cxp = mid.tile([PT, W], f32)
                cxq = nc.vector if cxp_eng == "dve" else nc.gpsimd
                cxq.tensor_tensor(cxp[:], v14[:, 0, :], pxt[:], op=AluOpType.add)
                cyp = mid.tile([PT, W], f32)
                nc.scalar.activation(cyp[:], v14[:, 1, :], Act.Identity, bias=pycol, scale=1.0)

                bb32 = scr.tile([PT, 4 * W], f32)
                bb4 = bb32.rearrange("p (w c) -> p w c", c=4)
                nc.vector.scalar_tensor_tensor(
                    bb4[:, :, 0], v14[:, 2, :], -0.5, cxp[:], AluOpType.mult, AluOpType.add)
                nc.vector.scalar_tensor_tensor(
                    bb4[:, :, 2], v14[:, 2, :], 0.5, cxp[:], AluOpType.mult, AluOpType.add)
                nc.vector.scalar_tensor_tensor(
                    bb4[:, :, 1], v14[:, 3, :], -0.5, cyp[:], AluOpType.mult, AluOpType.add)
                nc.vector.scalar_tensor_tensor(
                    bb4[:, :, 3], v14[:, 3, :], 0.5, cyp[:], AluOpType.mult, AluOpType.add)

                lm32 = scr.tile([PT, 10 * W], f32)
                lmi = lm32.rearrange("p (w j) -> p w j", j=10)
                olx = lmi[:, :, 0:10:2].rearrange("p w j -> p j w")
                oly = lmi[:, :, 1:10:2].rearrange("p w j -> p j w")
                lmx_src = l8[:, 0:5, :] if fp8 else v14[:, 4:9, :]
                lmy_src = l8[:, 5:10, :] if fp8 else v14[:, 9:14, :]
                nc.gpsimd.tensor_tensor(olx, lmx_src, pxb, op=AluOpType.add)
                nc.scalar.activation(oly, lmy_src, Act.Identity,
                                     bias=pycol, scale=1.0)

                # ---- masks (= f32 -> bf16 conversion) + store ----
                ot = outp.tile([PT, W * C], b16)
                ot4 = ot.rearrange("p (w c) -> p w c", c=C)
                nc.gpsimd.tensor_tensor(ot4[:, :, 0], sc[:], m[:], op=AluOpType.mult)
                halves = [slice(0, W // 2), slice(W // 2, W)] if last else [slice(0, W)]
                for ws in halves:
                    n = ws.stop - ws.start
                    nc.vector.tensor_tensor(
                        ot4[:, ws, 1:5], bb4[:, ws, :],
                        m[:, ws].broadcast_to([PT, n, 4]), op=AluOpType.mult)
                    nc.vector.tensor_tensor(
                        ot4[:, ws, 5:15], lmi[:, ws, :],
                        m[:, ws].broadcast_to([PT, n, 10]), op=AluOpType.mult)
                    stq.dma_start(out[b, r0:r0 + PT, ws, :], ot4[:, ws, :])

    nc.compile()
    return nc


def _np_h(mode=PROD_MODE):
    if mode.startswith("b16"):
        import ml_dtypes
        return ml_dtypes.bfloat16
    return np.float16


def _aux_inputs(mode=PROD_MODE):
    h = _np_h(mode)
    pxd = (np.arange(W, dtype=np.float32) * STRIDE + OFF_X)[None, :].repeat(PT, 0).astype(h)
    pyd = (np.arange(H, dtype=np.float32) * STRIDE + OFF_Y).reshape(NT, PT)
    return np.ascontiguousarray(pxd), np.ascontiguousarray(pyd)


def _in_maps_v4(x: np.ndarray):
    import ml_dtypes
    b16 = ml_dtypes.bfloat16
    x = np.asarray(x, dtype=np.float32)
    assert x.shape == (B, C, H, W), x.shape
    xs_full = np.ascontiguousarray(x[:, 0])
    # pair-interleave (dx,dy) and (sx,sy): [B, 2, H, 2W]
    xp = x[:, 1:5].reshape(B, 2, 2, H, W).transpose(0, 1, 3, 4, 2)
    xp2_full = np.ascontiguousarray(xp.reshape(B, 2, H, 2 * W).astype(b16))
    # landmarks planar, x-planes then y-planes: [B, 10, H, W]
    lm = x[:, 5:].reshape(B, 5, 2, H, W)
    xl_full = np.ascontiguousarray(
        np.concatenate([lm[:, :, 0], lm[:, :, 1]], axis=1).astype(b16))
    pxd = np.ascontiguousarray(
        (np.arange(W, dtype=np.float32) * STRIDE + OFF_X)[None, :].repeat(PT, 0))
    pyd = np.ascontiguousarray(
        (np.arange(H, dtype=np.float32) * STRIDE + OFF_Y).reshape(NT, PT))
    px2 = np.zeros((PT, 2 * W), np.float32)
    px2[:, 0::2] = pxd
    px2 = np.ascontiguousarray(px2.astype(b16))
    sy2 = np.zeros((PT, 2 * W), np.float32)
    sy2[:, 1::2] = 1.0
    sy2 = np.ascontiguousarray(sy2.astype(b16))
    return [
        {
            "xs": xs_full[i * B_LOCAL:(i + 1) * B_LOCAL],
            "xp2": xp2_full[i * B_LOCAL:(i + 1) * B_LOCAL],
            "xl": xl_full[i * B_LOCAL:(i + 1) * B_LOCAL],
            "pxd": pxd, "pyd": pyd, "px2d": px2, "sy2d": sy2,
        }
        for i in range(N_CORES)
    ]


def _in_maps_v6(x: np.ndarray, fp8: bool = False):
    import ml_dtypes
    b16 = ml_dtypes.bfloat16
    x = np.asarray(x, dtype=np.float32)
    assert x.shape == (B, C, H, W), x.shape
    xs_full = np.ascontiguousarray(x[:, 0])
    lm = x[:, 5:].reshape(B, 5, 2, H, W)
    lm_planar = np.concatenate([lm[:, :, 0], lm[:, :, 1]], axis=1)
    pxd = np.ascontiguousarray(
        (np.arange(W, dtype=np.float32) * STRIDE + OFF_X)[None, :].repeat(PT, 0))
    pyd = np.ascontiguousarray(
        (np.arange(H, dtype=np.float32) * STRIDE + OFF_Y).reshape(NT, PT))
    if fp8:
        f8 = ml_dtypes.float8_e4m3
        xr_full = np.ascontiguousarray(x[:, 1:5].astype(b16))
        xl_full = np.ascontiguousarray(lm_planar.astype(f8))
        return [
            {
                "xs": xs_full[i * B_LOCAL:(i + 1) * B_LOCAL],
                "xr": xr_full[i * B_LOCAL:(i + 1) * B_LOCAL],
                "xl8": xl_full[i * B_LOCAL:(i + 1) * B_LOCAL],
                "pxd": pxd, "pyd": pyd,
            }
            for i in range(N_CORES)
        ]
    xr_full = np.ascontiguousarray(
        np.concatenate([x[:, 1:5], lm_planar], axis=1).astype(b16))
    return [
        {
            "xs": xs_full[i * B_LOCAL:(i + 1) * B_LOCAL],
            "xr": xr_full[i * B_LOCAL:(i + 1) * B_LOCAL],
            "pxd": pxd, "pyd": pyd,
        }
        for i in range(N_CORES)
    ]


# channel order in the packed fp8 tensor: dx,dy,sx,sy, lmx0..4, lmy0..4
_V12_CH = [1, 2, 3, 4, 5, 7, 9, 11, 13, 6, 8, 10, 12, 14]


def _in_maps_v12(x: np.ndarray):
    import ml_dtypes
    f8 = ml_dtypes.float8_e4m3
    x = np.asarray(x, dtype=np.float32)
    assert x.shape == (B, C, H, W), x.shape
    xs_full = np.ascontiguousarray(x[:, 0])
    xq_full = np.ascontiguousarray(
        x[:, _V12_CH].transpose(0, 2, 1, 3).astype(f8))  # [B, H, 14, W]
    pxd = np.ascontiguousarray(
        (np.arange(W, dtype=np.float32) * STRIDE + OFF_X)[None, :].repeat(PT, 0))
    pyd = np.ascontiguousarray(
        (np.arange(H, dtype=np.float32) * STRIDE + OFF_Y).reshape(NT, PT))
    # 0/1 partition-shift matrices for the PE-matmul row shift (v12m):
    # sup[m] = sc[m-1] (clamped), sdn[m] = sc[m+1] (clamped)
    smu = np.zeros((PT, PT), np.float32)
    smu[np.arange(0, PT - 1), np.arange(1, PT)] = 1.0
    smd = np.zeros((PT, PT), np.float32)
    smd[np.arange(1, PT), np.arange(0, PT - 1)] = 1.0
    se127 = np.zeros((1, PT), np.float32)
    se127[0, PT - 1] = 1.0
    eye = np.eye(PT, dtype=np.float32)
    sid8 = np.ascontiguousarray(eye.astype(f8))
    smh8 = np.ascontiguousarray((-0.5 * eye).astype(f8))
    sph8 = np.ascontiguousarray((0.5 * eye).astype(f8))
    sone16 = np.ones((1, PT), np.float16)
    px16d = np.ascontiguousarray(
        (np.arange(W, dtype=np.float32) * STRIDE + OFF_X)[None, :].astype(np.float16))
    return [
        {
            "xs": xs_full[i * B_LOCAL:(i + 1) * B_LOCAL],
            "xq": xq_full[i * B_LOCAL:(i + 1) * B_LOCAL],
            "pxd": pxd, "pyd": pyd, "smu": smu, "smd": smd,
            "se127": se127, "sid8": sid8, "smh8": smh8, "sph8": sph8,
            "sone16": sone16, "px16d": px16d,
        }
        for i in range(N_CORES)
    ]


def _postprocess_out(out: np.ndarray, mode=PROD_MODE) -> np.ndarray:
    """Per-core raw device output -> [B_LOCAL, H, W, C] float32."""
    if mode.startswith(("v12", "v13", "v14")):
        return np.asarray(out).reshape(B_LOCAL, H, C, W).transpose(
            0, 1, 3, 2).astype(np.float32)
    return np.asarray(out, dtype=np.float32).reshape(B_LOCAL, H, W, C)


def _in_maps(x: np.ndarray, mode=PROD_MODE):
    if mode.startswith(("v12", "v13", "v14")):
        return _in_maps_v12(x)
    if mode.startswith("v4"):
        return _in_maps_v4(x)
    if mode.startswith(("v6", "v7", "v8", "v9")):
        return _in_maps_v6(
            x, fp8=(mode in ("v6f8", "v7f8") or mode.startswith(("v8", "v9"))))
    x = np.asarray(x, dtype=np.float32)
    assert x.shape == (B, C, H, W), x.shape
    pxd, pyd = _aux_inputs(mode)
    xs_full = np.ascontiguousarray(x[:, 0])
    xr_full = np.ascontiguousarray(x[:, 1:]).astype(_np_h(mode))
    return [
        {
            "xs": xs_full[i * B_LOCAL:(i + 1) * B_LOCAL],
            "xr": xr_full[i * B_LOCAL:(i + 1) * B_LOCAL],
            "pxd": pxd,
            "pyd": pyd,
        }
        for i in range(N_CORES)
    ]


def kernel(x: np.ndarray) -> np.ndarray:
    from concourse.bass_utils import run_bass_kernel_spmd

    if "nc" not in _CACHE:
        _CACHE["nc"] = _build_nc()
    nc = _CACHE["nc"]

    res = run_bass_kernel_spmd(nc, _in_maps(x), list(range(N_CORES)))
    return np.concatenate(
        [_postprocess_out(res.results[i]["out"]) for i in range(N_CORES)], axis=0
    )

